# revision 1
# baseline (speedup 1.0000x reference)
"""Trainium2 Bass kernel for nn_CloMSFM (CloFormer-style mixed local-conv +
global-attention block). Data-parallel over batch: 16 images -> 8 NeuronCores,
2 images per core, no collectives. Everything SBUF-resident per image; all
matmuls in bf16 (fp32 accumulation in PSUM).

Layouts (per image, per core):
  x          : [256, 1024] channels-on-partitions (2 chunks of 128), f32 + bf16
  qkv branch : padded [128, 34, 34] per 128-ch block for the 3x3 depthwise conv
               (done as 9 diagonal matmuls on the TensorEngine)
  attention  : Q^T/K^T [4h x 32d on partitions, 1024 tokens free];
               S^T per m-chunk via 4 head-packed K=32 matmuls (tile_position);
               U = exp(SCALOR*S^T) in bf16; numerator via [V | ones] lhsT so the
               softmax denominator falls out as matmul row 32.
"""

import os
import sys

for _p in ("/opt/trn_rl_repo",):
    if os.path.isdir(_p) and _p not in sys.path:
        sys.path.insert(0, _p)

import numpy as np
import ml_dtypes

BF16 = ml_dtypes.bfloat16

DIM = 256
DIM_HEAD = 32
SCALOR = DIM_HEAD ** (-0.5)
HF_CH = 128
QKV_CH = 384
LF_HEADS = 4
HW = 1024
B_PER_CORE = 2
N_CORES = 8

# ---- bf16 weight bundle column offsets (per 128-partition rows) ----
OF_QKV = 0            # 768   wqkvT: [kc][blk*128+m]
OF_A1 = 768           # 128
OF_A2 = 896           # 128   (0.5*act_w2).T
OF_GQ = 1024          # 256
OF_GK = 1280          # 256
OF_GV = 1536          # 256
OF_CA1 = 1792         # 32    (ca_w1/1024).T
OF_CA2 = 1824         # 256   rows 0-15
OF_ONES = 2080        # 128   row 0
OF_GVB = 2208         # 128   row 0
WB = 2336

# ---- f32 bundle ----
OF_QKVB = 0           # 3
OF_DWB = 3            # 3
OF_B1H = 6            # 1  0.5*act_b1
OF_B2S = 7            # 1  SCALOR*act_b2
OF_GQB = 8            # 1
OF_GKB = 9            # 1
OF_B1 = 10            # 1  act_b1 (full)
OF_PJB = 11           # 2
OF_WPJ = 13           # 512  proj_w.T in f32 (scaled by SE gate on-device)
WF = 525

_CACHE = {}


def _build_nc():
    import concourse.bass as bass
    import concourse.tile as tile
    from concourse import mybir, bacc

    f32 = mybir.dt.float32
    bf16 = mybir.dt.bfloat16
    AF = mybir.ActivationFunctionType
    OP = mybir.AluOpType

    nc = bacc.Bacc("TRN2", target_bir_lowering=False, debug=False,
                   num_devices=N_CORES, enable_asserts=True)

    xf_d = nc.dram_tensor("xf", [2, 2, 128, HW], f32, kind="ExternalInput").ap()
    xb_d = nc.dram_tensor("xb", [2, 2, 128, HW], bf16, kind="ExternalInput").ap()
    wb_d = nc.dram_tensor("wb", [128, WB], bf16, kind="ExternalInput").ap()
    wf_d = nc.dram_tensor("wf", [128, WF], f32, kind="ExternalInput").ap()
    dwv_d = nc.dram_tensor("dwv", [128, 27], f32, kind="ExternalInput").ap()
    out_d = nc.dram_tensor("out", [2, 2, 128, HW], f32, kind="ExternalOutput").ap()

    from contextlib import ExitStack
    with tile.TileContext(nc) as tc, ExitStack() as ctx:
        cw = ctx.enter_context(tc.tile_pool(name="cw", bufs=1))
        sb = ctx.enter_context(tc.tile_pool(name="sb", bufs=2))
        su = ctx.enter_context(tc.tile_pool(name="su", bufs=10))
        ps = ctx.enter_context(tc.tile_pool(name="ps", bufs=2, space="PSUM"))

        wb = cw.tile([128, WB], bf16, name="wb", tag="wb")
        nc.sync.dma_start(wb[:, :], wb_d[:, :])
        wf = cw.tile([128, WF], f32, name="wf", tag="wf")
        nc.sync.dma_start(wf[:, :], wf_d[:, :])
        dwv = cw.tile([128, 27], f32, name="dwv", tag="dwv")
        nc.sync.dma_start(dwv[:, :], dwv_d[:, :])
        from concourse.masks import make_identity
        dwdiag = cw.tile([128, 27, 128], bf16, name="dwdiag", tag="dwdiag")

        def emit_diag():
            ident = cw.tile([128, 128], bf16, name="ident", tag="ident")
            make_identity(nc, ident[:, :])
            for t27 in range(27):
                nc.vector.tensor_scalar_mul(dwdiag[:, t27, :], ident[:, :],
                                            dwv[:, t27:t27 + 1])

        def bias(off):  # [128,1] f32 view into the const bundle
            return wf[:, off:off + 1]

        st = [dict() for _ in range(2)]  # per-image tile state

        def phase_A_attn(img):
            s = st[img]
            xbc = [sb.tile([128, HW], bf16, name="xb", tag="xb") for _ in range(2)]
            for c in range(2):
                nc.sync.dma_start(xbc[c][:, :], xb_d[img, c])
            s["xb"] = xbc
            pf = sb.tile([128, 2], f32, name="pf", tag="pf")
            s["pf"] = pf
            yield
            gp = ps.tile([128, 2, 512], f32, name="mm", tag="mm", bufs=2)
            for nh in range(2):
                for kc in range(2):
                    nc.tensor.matmul(gp[:, nh, :],
                                     wb[:, OF_GQ + kc * 128:OF_GQ + kc * 128 + 128],
                                     xbc[kc][:, nh * 512:nh * 512 + 512],
                                     start=(kc == 0), stop=(kc == 1))
            qT = sb.tile([128, HW], bf16, name="qT", tag="qT")
            s["qT"] = qT
            nc.vector.tensor_scalar_add(qT.rearrange("p (a b) -> p a b", a=2),
                                        gp[:, 0:2, :], bias(OF_GQB))
            yield
            gk = ps.tile([128, 2, 512], f32, name="mm", tag="mm", bufs=2)
            for nh in range(2):
                for kc in range(2):
                    nc.tensor.matmul(gk[:, nh, :],
                                     wb[:, OF_GK + kc * 128:OF_GK + kc * 128 + 128],
                                     xbc[kc][:, nh * 512:nh * 512 + 512],
                                     start=(kc == 0), stop=(kc == 1))
            kT = sb.tile([128, HW], bf16, name="kT", tag="kT")
            s["kT"] = kT
            nc.vector.tensor_scalar_add(kT.rearrange("p (a b) -> p a b", a=2),
                                        gk[:, 0:2, :], bias(OF_GKB))
            yield

        def phase_A_conv(img):
            s = st[img]
            xbc, pf = s["xb"], s["pf"]
            vatt = sb.tile([128, 8, 4, 33], bf16, name="vatt", tag="vatt")
            s["vatt"] = vatt
            nc.gpsimd.memset(vatt[:, :, :, 32:33], 1.0)
            for mc in range(8):
                vp = ps.tile([128, 2, 512], f32, name="mm", tag="mm", bufs=2)
                for kc in range(2):
                    nc.tensor.matmul(vp[:, 0, 0:128],
                                     xbc[kc][:, mc * 128:mc * 128 + 128],
                                     wb[:, OF_GV + kc * 128:OF_GV + kc * 128 + 128],
                                     start=(kc == 0), stop=False)
                nc.tensor.matmul(vp[:, 0, 0:128], wb[0:1, OF_ONES:OF_ONES + 128],
                                 wb[0:1, OF_GVB:OF_GVB + 128], start=False, stop=True)
                nc.vector.tensor_copy(
                    vatt[:, mc, :, 0:32],
                    vp[:, 0, 0:128].rearrange("p (h d) -> p h d", d=32))
                yield

            pads = []
            for blk in range(3):
                pad = sb.tile([128, 34, 34], bf16, tag=f"pad{blk}")
                pads.append(pad)
                nc.gpsimd.memset(pad[:, 0, :], 0.0)
                nc.gpsimd.memset(pad[:, 33, :], 0.0)
                nc.gpsimd.memset(pad[:, 1:33, 0:1], 0.0)
                nc.gpsimd.memset(pad[:, 1:33, 33:34], 0.0)
                cps = ps.tile([128, 2, 512], f32, name="mm", tag="mm", bufs=2)
                for nh in range(2):
                    for kc in range(2):
                        nc.tensor.matmul(
                            cps[:, nh, :],
                            wb[:, OF_QKV + kc * 384 + blk * 128:OF_QKV + kc * 384 + blk * 128 + 128],
                            xbc[kc][:, nh * 512:nh * 512 + 512],
                            start=(kc == 0), stop=(kc == 1))
                src2 = cps[:, 0:2, :].rearrange("p a b -> p (a b)").rearrange(
                    "p (h w) -> p h w", w=32)
                nc.vector.tensor_scalar_add(pad[:, 1:33, 1:33], src2,
                                            bias(OF_QKVB + blk))
                yield

            dps = []
            for blk in range(3):
                dp = ps.tile([128, 2, 512], f32, name="mm", tag="mm", bufs=2)
                dps.append(dp)
                for nh in range(2):
                    for tap in range(9):
                        dh, dw = tap // 3, tap % 3
                        nc.tensor.matmul(
                            dp[:, nh, :],
                            dwdiag[:, blk * 9 + tap, :],
                            pads[blk][:, dh + 16 * nh:dh + 16 * nh + 16, dw:dw + 32],
                            start=(tap == 0), stop=(tap == 8))
                        if tap % 3 == 2:
                            yield

            q_sb = sb.tile([128, HW], bf16, name="q", tag="q")
            nc.vector.tensor_scalar_add(
                q_sb.rearrange("p (a b) -> p a b", a=2), dps[0][:, 0:2, :],
                bias(OF_DWB + 0))
            qk = sb.tile([128, HW], bf16, name="qk", tag="qk")
            nc.vector.scalar_tensor_tensor(
                qk.rearrange("p (a b) -> p a b", a=2), dps[1][:, 0:2, :],
                bias(OF_DWB + 1), q_sb.rearrange("p (a b) -> p a b", a=2),
                OP.add, OP.mult)
            v_sb = sb.tile([128, HW], bf16, name="v", tag="v")
            nc.vector.tensor_scalar_add(
                v_sb.rearrange("p (a b) -> p a b", a=2), dps[2][:, 0:2, :],
                bias(OF_DWB + 2))
            yield

            ap1 = ps.tile([128, 2, 512], f32, name="mm", tag="mm", bufs=2)
            for nh in range(2):
                nc.tensor.matmul(ap1[:, nh, :], wb[:, OF_A1:OF_A1 + 128],
                                 qk[:, nh * 512:nh * 512 + 512], start=True, stop=True)
            t_sb = sb.tile([128, HW], bf16, name="t", tag="t")
            nc.scalar.activation(t_sb.rearrange("p (a b) -> p a b", a=2),
                                 ap1[:, 0:2, :], AF.Tanh, bias=bias(OF_B1H), scale=0.5)
            a_sb = sb.tile([128, HW], bf16, name="a", tag="a")
            nc.vector.tensor_scalar_add(
                a_sb.rearrange("p (a b) -> p a b", a=2), ap1[:, 0:2, :],
                bias(OF_B1))
            sw = sb.tile([128, HW], bf16, name="sw", tag="sw")
            nc.vector.scalar_tensor_tensor(sw[:, :], t_sb[:, :], 1.0, a_sb[:, :],
                                           OP.add, OP.mult)
            yield

            ap2 = ps.tile([128, 2, 512], f32, name="mm", tag="mm", bufs=2)
            for nh in range(2):
                nc.tensor.matmul(ap2[:, nh, :], wb[:, OF_A2:OF_A2 + 128],
                                 sw[:, nh * 512:nh * 512 + 512], start=True, stop=True)
            th = sb.tile([128, HW], bf16, name="th", tag="th")
            nc.scalar.activation(th.rearrange("p (a b) -> p a b", a=2),
                                 ap2[:, 0:2, :], AF.Tanh, bias=bias(OF_B2S),
                                 scale=SCALOR)
            comb = sb.tile([128, 2, HW], bf16, name="comb", tag="comb")
            s["comb"] = comb
            nc.vector.scalar_tensor_tensor(comb[:, 0, :], th[:, :], 1.0, v_sb[:, :],
                                           OP.mult, OP.mult, accum_out=pf[:, 0:1])
            yield

        def phase_S(img):
            s = st[img]
            qT, kT = s["qT"], s["kT"]
            u_list = []
            s["u"] = u_list
            for j in range(8):
                ut = su.tile([128, 4, HW], bf16, name="u", tag="u")
                u_list.append(ut)
                for nh in range(2):
                    for g in range(2):
                        sp = ps.tile([128, 2, 512], f32, name="sp", tag="sp", bufs=2)
                        for hh in range(2):
                            h = 2 * g + hh
                            nc.tensor.matmul(
                                sp[:, hh, :],
                                kT[32 * h:32 * h + 32, j * 128:j * 128 + 128],
                                qT[32 * h:32 * h + 32, nh * 512:nh * 512 + 512],
                                start=True, stop=True, tile_position=(32 * h, 0))
                        nc.scalar.activation(
                            ut[:, 2 * g:2 * g + 2, nh * 512:nh * 512 + 512],
                            sp[:, :, :], AF.Exp, scale=SCALOR)
                        yield

        def phase_V(img):
            s = st[img]
            vatt, u_list, comb, pf = s["vatt"], s["u"], s["comb"], s["pf"]
            for h in range(4):
                vp = ps.tile([128, 2, 512], f32, name="vmm", tag="mm", bufs=2)
                for nh in range(2):
                    for kc in range(8):
                        nc.tensor.matmul(
                            vp[0:33, nh, :], vatt[:, kc, h, :],
                            u_list[kc][:, h, nh * 512:nh * 512 + 512],
                            start=(kc == 0), stop=(kc == 7))
                yield
                den_sb = sb.tile([1, HW], f32, name="den_sb", tag="den_sb")
                nc.scalar.activation(
                    den_sb[0:1, :].rearrange("p (a b) -> p a b", a=2),
                    vp[32:33, 0:2, :], AF.Identity)
                rr = sb.tile([1, HW], f32, name="rr", tag="rr")
                nc.vector.reciprocal_approx_fast(rr[0:1, :], den_sb[0:1, :])
                rrb = sb.tile([1, HW], bf16, name="rrb", tag="rrb")
                nc.scalar.activation(rrb[:, :], rr[:, :], AF.Identity)
                ep = ps.tile([128, 2, 512], f32, name="ep", tag="sp", bufs=2)
                for nh in range(2):
                    nc.tensor.matmul(ep[0:32, nh, :],
                                     wb[0:1, OF_ONES:OF_ONES + 32],
                                     rrb[0:1, nh * 512:nh * 512 + 512],
                                     start=True, stop=True)
                rbc = sb.tile([32, HW], f32, name="rbc", tag="rbc")
                nc.scalar.activation(
                    rbc[0:32, :].rearrange("p (a b) -> p a b", a=2),
                    ep[0:32, 0:2, :], AF.Identity)
                nc.vector.scalar_tensor_tensor(
                    comb[32 * h:32 * h + 32, 1, :].rearrange("p (a b) -> p a b", a=2),
                    vp[0:32, 0:2, :], 1.0,
                    rbc[0:32, :].rearrange("p (a b) -> p a b", a=2),
                    OP.mult, OP.mult,
                    accum_out=pf[32 * h:32 * h + 32, 1:2])
                yield

        def phase_P(img):
            s = st[img]
            comb, pf = s["comb"], s["pf"]
            xfc = [sb.tile([128, HW], f32, name="xf", tag="xf") for _ in range(2)]
            for c in range(2):
                nc.sync.dma_start(xfc[c][:, :], xf_d[img, c])
            pb = sb.tile([128, 2], bf16, name="pb", tag="pb")
            nc.vector.tensor_copy(pb[:, :], pf[:, :])
            zp = ps.tile([128, 2, 512], f32, name="mm", tag="mm", bufs=2)
            for kc in range(2):
                nc.tensor.matmul(zp[0:16, 0, 0:1],
                                 wb[:, OF_CA1 + kc * 16:OF_CA1 + kc * 16 + 16],
                                 pb[:, kc:kc + 1], start=(kc == 0), stop=(kc == 1))
            z1r = sb.tile([16, 1], bf16, name="z1r", tag="z1r")
            nc.scalar.activation(z1r[:, :], zp[0:16, 0, 0:1], AF.Relu)
            zp2 = ps.tile([128, 2, 512], f32, name="mm2", tag="mm", bufs=2)
            for mc in range(2):
                nc.tensor.matmul(zp2[:, mc, 0:1],
                                 wb[0:16, OF_CA2 + mc * 128:OF_CA2 + mc * 128 + 128],
                                 z1r[:, :], start=True, stop=True)
            tse = sb.tile([128, 2], f32, name="tse", tag="tse")
            nc.scalar.activation(tse[:, :], zp2[:, 0:2, 0], AF.Tanh, scale=0.5)
            ca = sb.tile([128, 2], f32, name="ca", tag="ca")
            nc.vector.tensor_scalar(ca[:, :], tse[:, :], 0.5, 0.5,
                                    OP.mult, OP.add)
            wps = sb.tile([128, 2, 256], bf16, name="wps", tag="wps")
            for kc in range(2):
                nc.vector.tensor_scalar_mul(wps[:, kc, :],
                                            wf[:, OF_WPJ + kc * 256:OF_WPJ + kc * 256 + 256],
                                            ca[:, kc:kc + 1])
            yield
            for mc in range(2):
                pp = ps.tile([128, 2, 512], f32, name="mm3", tag="mm", bufs=2)
                for nh in range(2):
                    for kc in range(2):
                        nc.tensor.matmul(pp[:, nh, :],
                                         wps[:, kc, mc * 128:mc * 128 + 128],
                                         comb[:, kc, nh * 512:nh * 512 + 512],
                                         start=(kc == 0), stop=(kc == 1))
                ot = sb.tile([128, HW], f32, name="o", tag="o")
                nc.vector.scalar_tensor_tensor(
                    ot.rearrange("p (a b) -> p a b", a=2), pp[:, 0:2, :],
                    bias(OF_PJB + mc),
                    xfc[mc].rearrange("p (a b) -> p a b", a=2),
                    OP.add, OP.add)
                nc.sync.dma_start(out_d[img, mc], ot[:, :])
                yield

        def run(gen):
            for _ in gen:
                pass

        def weave(*gens):
            active = list(gens)
            while active:
                for g in list(active):
                    try:
                        next(g)
                    except StopIteration:
                        active.remove(g)

        def chain(*gens):
            for g in gens:
                yield from g

        run(phase_A_attn(0))
        emit_diag()
        weave(phase_S(0), chain(phase_A_conv(0), phase_A_attn(1)))
        weave(phase_S(1), chain(phase_V(0), phase_A_conv(1), phase_P(0)))
        run(phase_V(1))
        run(phase_P(1))

    nc.compile()
    return nc


def _prep_weights(i):
    """Host-side preprocessing -> (wb [128,WB] bf16, wf [128,WF] f32)."""
    wb = np.zeros((128, WB), np.float32)
    wf = np.zeros((128, WF), np.float32)
    p = np.arange(128)

    qkv_w = i["qkv_w"]          # [384, 256]
    for kc in range(2):
        for blk in range(3):
            wb[:, OF_QKV + kc * 384 + blk * 128:OF_QKV + kc * 384 + (blk + 1) * 128] = \
                qkv_w[blk * 128:(blk + 1) * 128, kc * 128:(kc + 1) * 128].T
    dw = i["dw_w"].reshape(QKV_CH, 3, 3)   # [384, 3, 3]
    dwv = np.zeros((128, 27), np.float32)
    for blk in range(3):
        for tap in range(9):
            dwv[:, blk * 9 + tap] = dw[blk * 128:(blk + 1) * 128, tap // 3, tap % 3]
    wb[:, OF_A1:OF_A1 + 128] = i["act_w1"].T
    wb[:, OF_A2:OF_A2 + 128] = (0.5 * i["act_w2"]).T
    for kc in range(2):
        wb[:, OF_GQ + kc * 128:OF_GQ + (kc + 1) * 128] = i["gq_w"][:, kc * 128:(kc + 1) * 128].T
        wb[:, OF_GK + kc * 128:OF_GK + (kc + 1) * 128] = i["gkv_w"][0:128, kc * 128:(kc + 1) * 128].T
        wb[:, OF_GV + kc * 128:OF_GV + (kc + 1) * 128] = i["gkv_w"][128:256, kc * 128:(kc + 1) * 128].T
        wb[:, OF_CA1 + kc * 16:OF_CA1 + (kc + 1) * 16] = (i["ca_w1"][:, kc * 128:(kc + 1) * 128] / HW).T
    wb[0:16, OF_CA2:OF_CA2 + 256] = i["ca_w2"].T[0:16, :]
    wb[0:1, OF_ONES:OF_ONES + 128] = 1.0
    wb[0:1, OF_GVB:OF_GVB + 128] = i["gkv_b"][128:256]

    for blk in range(3):
        wf[:, OF_QKVB + blk] = i["qkv_b"][blk * 128:(blk + 1) * 128]
        wf[:, OF_DWB + blk] = i["dw_b"][blk * 128:(blk + 1) * 128]
    wf[:, OF_B1H] = 0.5 * i["act_b1"]
    wf[:, OF_B2S] = SCALOR * i["act_b2"]
    wf[:, OF_GQB] = i["gq_b"]
    wf[:, OF_GKB] = i["gkv_b"][0:128]
    wf[:, OF_B1] = i["act_b1"]
    for mc in range(2):
        wf[:, OF_PJB + mc] = i["proj_b"][mc * 128:(mc + 1) * 128]
    for kc in range(2):
        wf[:, OF_WPJ + kc * 256:OF_WPJ + (kc + 1) * 256] = i["proj_w"][:, kc * 128:(kc + 1) * 128].T

    return wb.astype(BF16), wf, dwv


def kernel(**inputs):
    from concourse.bass_utils import run_bass_kernel_spmd

    i = {k: np.asarray(v, np.float32) for k, v in inputs.items()}
    if "nc" not in _CACHE:
        _CACHE["nc"] = _build_nc()
    nc = _CACHE["nc"]

    wb, wf, dwv = _prep_weights(i)
    x = i["x"].reshape(16, 256, HW)
    in_maps = []
    for c in range(N_CORES):
        xs = x[c * B_PER_CORE:(c + 1) * B_PER_CORE].reshape(2, 2, 128, HW)
        in_maps.append({
            "xf": np.ascontiguousarray(xs),
            "xb": np.ascontiguousarray(xs.astype(BF16)),
            "wb": wb, "wf": wf, "dwv": dwv,
        })
    res = run_bass_kernel_spmd(nc, in_maps, core_ids=list(range(N_CORES)))
    out = np.stack([r["out"] for r in res.results])  # [8, 2, 2, 128, HW]
    return out.reshape(16, 256, 32, 32).astype(np.float32)

